# revision 36
# baseline (speedup 1.0000x reference)
"""Multi-head dot-product attention with prefix KV, on 8 trn2 NeuronCores.

Sharding: batch (2) x head-groups (4 groups of 4 heads) = 8 cores.
Each core computes q/k/v projections for its 4 heads, flash-style
attention (scores kept transposed: [kv, L] so no on-device transposes
are needed), and a partial out-projection [E, L]; the host sums the 4
head-group partials per batch and transposes back.

Device-side layout notes:
  - All matmul operands are bf16 (PSUM accumulation stays fp32): on HW,
    f32r lowers to fp32_mode=HIGH multi-pass matmuls (~2-4 cyc/row) and
    2-pass LDWEIGHTS; bf16 runs 1 cyc/row with half-cost weight loads.
  - Host pre-transposes inputs_q/inputs_kv to x^T [E, L] so both the
    qT/kT projections (lhsT=W) and the natural-layout v projection
    (lhsT=x^T tiles) need no on-device transposes.
  - kv axis is padded to 2176 = 17*128: chunk 0 = [prefix(64) | dead(64)],
    chunks 1..16 = kv positions.  Dead columns are killed with a
    per-partition -1e10 bias on the chunk-0 exp.
  - softmax runs without max subtraction (scores are O(1); masked
    entries underflow to exactly 0 like the reference's -1e10 bias).
  - denominator comes free as an extra ones-column in the v weights
    (M=65 ctx matmul); denominator rows of all 4 heads are gathered to
    partitions {0,32,64,96} of one tile, approx-reciprocal'd in one DVE
    op, and broadcast across partitions with K=1 outer-product matmuls.
  - q/k projections use 512-wide moving dims (1 PSUM bank) to amortize
    LDWEIGHTS; all projection / out-projection units are injected as PE
    filler BETWEEN the score matmuls and the ctx matmuls of every
    attention batch, covering the softmax (ACT) latency in the in-order
    PE queue for all four L-groups.
"""

import numpy as np

B, LQ, LKV, E, H, D, P = 2, 2048, 2048, 1024, 16, 64, 64
NCORES = 8
HGROUPS = 4          # head groups (cores per batch)
HPC = H // HGROUPS   # heads per core = 4
KVPAD = 128 + LKV    # 2176
NCH = KVPAD // 128   # 17 chunks
NG = LQ // 512       # 4 L-groups of 512
NEG = -1.0e10

_CACHE = {}


def _build_module(plan, debug_taps=False):
    """Build the single-core Bass module (same program for all 8 cores)."""
    import concourse.bass as bass
    import concourse.tile as tile
    import concourse.mybir as mybir
    from concourse import bacc
    from contextlib import ExitStack

    f32 = mybir.dt.float32
    bf16 = mybir.dt.bfloat16
    Exp = mybir.ActivationFunctionType.Exp

    chunks, mixed_idx, nmix = plan["chunks"], plan["mixed_idx"], plan["nmix"]

    nc = bacc.Bacc("TRN2", target_bir_lowering=False, debug=False,
                   enable_asserts=False, num_devices=NCORES)

    # x and weights are pre-arranged on the host so every big DMA is a
    # dense per-partition-contiguous 2D transfer (no strided gathers).
    xqT_d = nc.dram_tensor("xqT", [NG, 128, 8, 512], bf16, kind="ExternalInput").ap()
    xkvT_d = nc.dram_tensor("xkvT", [NG, 128, 8, 512], bf16, kind="ExternalInput").ap()
    wq_d = nc.dram_tensor("wq", [128, 8, 256], bf16, kind="ExternalInput").ap()
    wk_d = nc.dram_tensor("wk", [128, 8, 256], bf16, kind="ExternalInput").ap()
    wv_d = nc.dram_tensor("wv", [128, 8, 256], bf16, kind="ExternalInput").ap()
    wo_d = nc.dram_tensor("wo", [128, 2, 1024], bf16, kind="ExternalInput").ap()
    kprefT_d = nc.dram_tensor("kprefT", [2, 128, 128], bf16, kind="ExternalInput").ap()
    vpref_d = nc.dram_tensor("vpref", [128, HPC, D], bf16, kind="ExternalInput").ap()
    if nmix:
        maskblk_d = nc.dram_tensor("maskblk", [nmix, 128, 512], bf16,
                                   kind="ExternalInput").ap()
    # bf16 output partials: halves the store drain; the host sums the four
    # per-core partials in fp32 (adds ~1e-3 abs error, well inside budget).
    outT_d = nc.dram_tensor("outT", [E, LQ], bf16, kind="ExternalOutput").ap()

    with tile.TileContext(nc) as tc, ExitStack() as stk:
        pers = stk.enter_context(tc.tile_pool(name="pers", bufs=1))

        def ptile(shape, name, dt=None):
            return pers.tile(shape, dt or bf16, tag=name, name=name)

        wq_sb = ptile([128, 8, 256], "wq_sb")
        wk_sb = ptile([128, 8, 256], "wk_sb")
        wv_sb = ptile([128, 8, 256], "wv_sb")
        wo_sb = ptile([128, 2, 1024], "wo_sb")
        # per-slice tensors: QTS[hc][g] 512-wide; KTS[hc][s]: s=0 prefix
        # [128,128], s>=1 [128,256] (kv chunks 2s-1, 2s); VTS[c] per chunk.
        QTS = [[ptile([128, 512], f"QT{i}g{g}") for g in range(NG)] for i in range(2)]
        KTS = [[ptile([128, 128] if s == 0 else [128, 256], f"KT{i}s{s}")
                for s in range(9)] for i in range(2)]
        VTS = [ptile([128, HPC, 65], f"VT{c}") for c in range(NCH)]
        CTXT = [[ptile([128, 512], f"CTXT{i}g{g}") for g in range(NG)]
                for i in range(2)]
        cb0 = ptile([128, 1], "cb0", f32)
        ones_col = ptile([128, 64], "ones_col")

        def kslice(hc, c):
            if c == 0:
                return KTS[hc][0][:, 0:128]
            s, off = (c + 1) // 2, 128 * ((c - 1) % 2)
            return KTS[hc][s][:, off:off + 128]

        # wq goes first on the sync hardware DGE queue (whose dispatch comes
        # up ~2.5us before the scalar engine's at boot), in parallel with the
        # first x load on the scalar queue; remaining weights follow after
        # the first x loads so the critical first projection isn't queued
        # behind 2MB of other transfers.
        nc.sync.dma_start(out=wq_sb, in_=wq_d)

        def load_rest():
            nc.scalar.dma_start(out=wk_sb, in_=wk_d)
            nc.sync.dma_start(out=wv_sb, in_=wv_d)
            nc.scalar.dma_start(out=wo_sb, in_=wo_d)
            for hc in range(2):
                nc.sync.dma_start(out=KTS[hc][0], in_=kprefT_d[hc])
            nc.sync.dma_start(out=VTS[0][:, :, 0:D], in_=vpref_d)

        nc.vector.memset(cb0[0:64, :], 0.0)
        nc.vector.memset(cb0[64:128, :], NEG)
        nc.gpsimd.memset(ones_col, 1.0)
        for c in range(NCH):
            nc.gpsimd.memset(VTS[c][:, :, 64:65], 1.0)

        xio = stk.enter_context(tc.tile_pool(name="xio", bufs=2))
        attps = stk.enter_context(tc.tile_pool(name="att_ps", bufs=1, space="PSUM"))
        attsb = stk.enter_context(tc.tile_pool(name="att_sb", bufs=1))
        pp = stk.enter_context(tc.tile_pool(name="pp", bufs=1, space="PSUM"))

        def proj_load(g):
            xq_t = xio.tile([128, 8, 512], bf16, tag="xq", bufs=2, name="xq_t")
            xkv_t = xio.tile([128, 8, 512], bf16, tag="xkv", bufs=2, name="xkv_t")
            nc.sync.dma_start(out=xq_t, in_=xqT_d[g])
            nc.scalar.dma_start(out=xkv_t, in_=xkvT_d[g])
            return xq_t, xkv_t

        def q_unit(g, t, xq_t):
            """qT projection for 512-wide L-group g, 128-row chunk t."""
            ps_q = pp.tile([128, 512], f32, tag="pj", bufs=2, name="ps_q")
            for ec in range(8):
                nc.tensor.matmul(
                    ps_q, lhsT=wq_sb[:, ec, 128 * t:128 * t + 128],
                    rhs=xq_t[:, ec, :], start=(ec == 0), stop=(ec == 7))
            nc.vector.tensor_copy(out=QTS[t][g], in_=ps_q)

        def k_unit(g, t, xkv_t):
            ps_k = pp.tile([128, 512], f32, tag="pj", bufs=2, name="ps_k")
            for ec in range(8):
                nc.tensor.matmul(
                    ps_k, lhsT=wk_sb[:, ec, 128 * t:128 * t + 128],
                    rhs=xkv_t[:, ec, :], start=(ec == 0), stop=(ec == 7))
            nc.vector.tensor_copy(out=KTS[t][2 * g + 1], in_=ps_k[:, 0:256])
            nc.vector.tensor_copy(out=KTS[t][2 * g + 2], in_=ps_k[:, 256:512])

        def v_unit(g, sub, xkv_t):
            """v projection for kv chunk 4g+sub+1 (natural [kv, hd] layout)."""
            ps_v = pp.tile([128, 512], f32, tag="pj", bufs=2, name="ps_v")
            for ec in range(8):
                nc.tensor.matmul(
                    ps_v[:, 0:256], lhsT=xkv_t[:, ec, 128 * sub:128 * sub + 128],
                    rhs=wv_sb[:, ec, :], start=(ec == 0), stop=(ec == 7))
            nc.vector.tensor_copy(
                out=VTS[1 + 4 * g + sub][:, :, 0:D],
                in_=ps_v[:, 0:256].rearrange("p (h d) -> p h d", h=HPC))

        def proj_units(g, loaded, skip_v=False):
            """Generator of (pe_cost_us, closure) projection units for group g."""
            xq_t, xkv_t = loaded
            for t in range(2):
                yield (1.7, lambda t=t: q_unit(g, t, xq_t))
                yield (1.7, lambda t=t: k_unit(g, t, xkv_t))
            if not skip_v:
                for sub in range(4):
                    yield (0.85, lambda sub=sub: v_unit(g, sub, xkv_t))

        def outproj_unit(g, et):
            gl = 512 * g
            ops = pp.tile([128, 512], f32, tag="pj", bufs=2, name="ops")
            for hc in range(2):
                nc.tensor.matmul(
                    ops, lhsT=wo_sb[:, hc, 128 * et:128 * et + 128],
                    rhs=CTXT[hc][g], start=(hc == 0), stop=(hc == 1))
            ot = attsb.tile([128, 512], bf16, tag="ostage", bufs=3, name="ot")
            # during attention (g<3) ACT is busy with exp: keep copies on DVE
            # and stores on the sync queue.  In the tail (g==3) ACT is idle:
            # alternate both the copy engine and the DMA queue to halve the
            # final drain.
            if g < 3 or et % 2 == 0:
                nc.vector.tensor_copy(out=ot, in_=ops)
            else:
                nc.scalar.copy(ot, ops)
            eng = nc.sync if (g < 3 or et % 2 == 0) else nc.scalar
            eng.dma_start(
                out=outT_d[128 * et:128 * et + 128, gl:gl + 512], in_=ot)

        def outproj_units(g):
            for et in range(8):
                yield (0.5, lambda et=et: outproj_unit(g, et))

        def attn_group(g, mts, filler=None):
            def fill(target=1.0):
                if filler is None:
                    return
                acc = 0.0
                while acc < target:
                    try:
                        cost, fn = next(filler)
                    except StopIteration:
                        return
                    fn()
                    acc += cost

            cs = chunks[g]
            batches = [[cs[0]]] + [cs[1 + i:3 + i] for i in range(0, len(cs) - 1, 2)]
            nb = len(batches)
            ctxs = {}
            pend = []

            def qstart(c):
                """First valid q column (within the group's 512) for chunk c.
                Diagonal chunks with jd>=1 have no valid scores for q<128*jd:
                those columns are skipped in scores/exp/ctx instead of being
                masked (the jd==0 chunk keeps the full-width mask multiply
                for its in-block triangle)."""
                if (g, c) in mixed_idx:
                    jd = c - 1 - 4 * g
                    if 1 <= jd <= 3:
                        return 128 * jd
                return 0
            for hp in range(HPC // 2):
                heads = (2 * hp, 2 * hp + 1)
                ctx_ps = {}
                for h in heads:
                    ctx_ps[h] = attps.tile([65, 512], f32, tag="ctx", bufs=2,
                                           name=f"ctx{h}")

                def emit_scores(bi, batch):
                    """Scores + exp + mask for one batch; returns pr tiles."""
                    w = 512 * len(batch)
                    w0 = qstart(batch[0])
                    pr = {}
                    for h in heads:
                        sc = attps.tile([128, 1024], f32, tag="sc", bufs=2,
                                        name=f"sc{h}")
                        for j, c in enumerate(batch):
                            prow = 64 * (h % 2)
                            q0 = qstart(c)
                            nc.tensor.matmul(
                                sc[:, 512 * j + q0:512 * j + 512],
                                lhsT=kslice(hp, c)[prow:prow + 64, :],
                                rhs=QTS[hp][g][prow:prow + 64, q0:512],
                                start=True, stop=True)
                        pr[h] = attsb.tile([128, 1024], bf16, tag="probs",
                                           bufs=4, name=f"pr{h}")
                        if batch[0] == 0:
                            nc.scalar.activation(pr[h][:, 0:w], sc[:, 0:w],
                                                 Exp, bias=cb0[:, 0:1])
                        else:
                            nc.scalar.activation(pr[h][:, w0:w], sc[:, w0:w], Exp)
                        for j, c in enumerate(batch):
                            if (g, c) in mts:
                                q0 = qstart(c)
                                if q0 == 0:
                                    nc.vector.tensor_mul(
                                        pr[h][:, 512 * j:512 * j + 512],
                                        pr[h][:, 512 * j:512 * j + 512],
                                        mts[(g, c)])
                                else:
                                    # only the 128-wide in-block triangle
                                    # needs masking; columns < q0 are never
                                    # read and columns > q0+128 are all-valid
                                    o = 512 * j + q0
                                    nc.vector.tensor_mul(
                                        pr[h][:, o:o + 128],
                                        pr[h][:, o:o + 128], mts[(g, c)])
                    return pr

                def emit_ctx(bi, batch, pr):
                    for h in heads:
                        for j, c in enumerate(batch):
                            last = (bi == nb - 1 and j == len(batch) - 1)
                            q0 = qstart(c)
                            V = VTS[c][:, h % 2 + 2 * hp, :]
                            if q0 == 0:
                                nc.tensor.matmul(
                                    ctx_ps[h], lhsT=V,
                                    rhs=pr[h][:, 512 * j:512 * j + 512],
                                    start=(bi == 0 and j == 0), stop=last)
                            else:
                                o = 512 * j + q0
                                nc.tensor.matmul(
                                    ctx_ps[h][:, q0:q0 + 128], lhsT=V,
                                    rhs=pr[h][:, o:o + 128],
                                    start=False, stop=last and q0 == 384,
                                    skip_group_check=True)
                                if q0 + 128 < 512:
                                    nc.tensor.matmul(
                                        ctx_ps[h][:, q0 + 128:512], lhsT=V,
                                        rhs=pr[h][:, o + 128:512 * j + 512],
                                        start=False, stop=last,
                                        skip_group_check=True)

                # software-pipelined emission: scores of batch bi+1 sit in the
                # in-order PE queue BEFORE ctx of batch bi, so the PE streams
                # scores(bi+1) while exp(bi) runs on ACT — attention hides its
                # own softmax latency; filler is only topping up.
                prev = None
                for bi, batch in enumerate(batches):
                    pr = emit_scores(bi, batch)
                    fill(0.5)
                    if pend and bi == 1:
                        # inject the previous head-pair's broadcast+scale here:
                        # by now its DVE reciprocal chain has resolved, so the
                        # PE does not stall on it.
                        for fn in pend:
                            fn()
                        pend = []
                    if prev is not None:
                        emit_ctx(*prev)
                    prev = (bi, batch, pr)
                emit_ctx(*prev)
                # head-pair epilogue: drain ctx PSUM, then per-head in-place
                # approx-reciprocal of the denominator row at partition 64
                # (no cross-partition DMA needed); the PE-side broadcast+scale
                # is deferred (pend) into the next head-pair's batch stream.
                rcf = attsb.tile([65, 1024], f32, tag="rcf", bufs=2, name="rcf")
                rcb = attsb.tile([65, 1024], bf16, tag="rcb", bufs=2, name="rcb")
                for h in heads:
                    ctxs[h] = attsb.tile([65, 512], f32, tag="ctxs", bufs=4,
                                         name=f"ctxs{h}")
                    nc.vector.tensor_copy(out=ctxs[h], in_=ctx_ps[h])
                    hr = h % 2
                    # full-tile raf: custom-DVE uops misbehave on partition
                    # slices with non-zero base, and the cost is free-size
                    # based anyway; only row 64 (the denom) is meaningful.
                    nc.vector.reciprocal_approx_fast(
                        out=rcf[:, 512 * hr:512 * hr + 512], in_=ctxs[h])
                nc.vector.tensor_copy(out=rcb[64:65, :], in_=rcf[64:65, :])

                def bcast_scale(hp=hp, heads=heads, rcb=rcb):
                    for h in heads:
                        hr = h % 2
                        bc_ps = pp.tile([64, 512], f32, tag="pj", bufs=2,
                                        name="bc_ps")
                        nc.tensor.matmul(bc_ps,
                                         lhsT=ones_col[64:65, :],
                                         rhs=rcb[64:65, 512 * hr:512 * hr + 512],
                                         start=True, stop=True,
                                         tile_position=(64, 0))
                        if hr == 0:
                            nc.vector.tensor_mul(CTXT[hp][g][0:64, :],
                                                 ctxs[h][0:64, :], bc_ps)
                        else:
                            st = attsb.tile([64, 512], bf16, tag="stage", bufs=2,
                                            name="st")
                            nc.vector.tensor_mul(st, ctxs[h][0:64, :], bc_ps)
                            # scalar queue: this partition move gates the
                            # group's out-projection, so don't queue it
                            # behind pending output stores on sync
                            nc.scalar.dma_start(out=CTXT[hp][g][64:128, :], in_=st)
                pend.append(bcast_scale)
            # last head-pair's chain: give the PE filler work while the DVE
            # reciprocal resolves, then emit the broadcast+scale.
            fill(3.0)
            for fn in pend:
                fn()

        # mask tiles loaded lazily per group so their DMA dispatches don't
        # delay the first x loads on the in-order sync engine.
        mts = {}

        def load_masks(g):
            for c in chunks[g]:
                if (g, c) in mixed_idx:
                    jd = c - 1 - 4 * g
                    if 1 <= jd <= 3:
                        # only the 128-wide in-block triangle is needed
                        mt = attsb.tile([128, 128], bf16, tag="mask",
                                        bufs=max(nmix, 1), name=f"mt{g}_{c}")
                        nc.sync.dma_start(
                            out=mt, in_=maskblk_d[mixed_idx[(g, c)]][:, 0:128])
                    else:
                        mt = attsb.tile([128, 512], bf16, tag="mask",
                                        bufs=max(nmix, 1), name=f"mt{g}_{c}")
                        nc.sync.dma_start(out=mt, in_=maskblk_d[mixed_idx[(g, c)]])
                    mts[(g, c)] = mt

        # interleaved schedule: group-g attention is fed PE-filler units
        # (projections of group g+1, then out-projections of finished
        # groups) between its scores and ctx matmuls.  Fill demand per
        # group is 2*len(batches)+2 pulls = 8/12/16/20; supply is matched
        # where possible.
        def chain(*gens):
            for gn in gens:
                yield from gn

        ld0 = proj_load(0)
        load_rest()
        load_masks(0)
        for _, fn in proj_units(0, ld0):
            fn()
        ld1 = proj_load(1)
        load_masks(1)
        f1 = proj_units(1, ld1)
        attn_group(0, mts, filler=f1)
        for _, fn in f1:
            fn()
        ld2 = proj_load(2)
        load_masks(2)
        op0 = list(outproj_units(0))
        op1 = list(outproj_units(1))
        f2 = chain(proj_units(2, ld2), iter(op0[:4]))
        attn_group(1, mts, filler=f2)
        for _, fn in f2:
            fn()
        ld3 = proj_load(3)
        load_masks(3)
        f3 = chain(iter(op0[4:]), proj_units(3, ld3, skip_v=True), iter(op1[:4]))
        attn_group(2, mts, filler=f3)
        for _, fn in f3:
            fn()
        xkv3 = ld3[1]
        f4 = chain(((0.85, lambda sub=sub: v_unit(3, sub, xkv3))
                    for sub in range(4)),
                   iter(op1[4:]), outproj_units(2))
        attn_group(3, mts, filler=f4)
        for _, fn in f4:
            fn()
        for _, fn in outproj_units(3):
            fn()

    nc.compile()
    return nc


def _make_plan(mask):
    """Block plan from the actual mask (union over batches -> one SPMD plan)."""
    m = np.asarray(mask[:, 0])                       # [B, LQ, LKV] bool
    blk = m.reshape(B, NG, 512, LKV // 128, 128)
    any_b = blk.any(axis=(2, 4)).any(axis=0)         # [NG, 16]
    all_b = blk.all(axis=(2, 4)).all(axis=0)         # [NG, 16]
    chunks, mixed_idx = [], {}
    order = []
    for g in range(NG):
        cl = [0]
        for c in range(1, NCH):
            if any_b[g, c - 1]:
                cl.append(c)
                if not all_b[g, c - 1]:
                    mixed_idx[(g, c)] = len(order)
                    order.append((g, c))
        chunks.append(cl)
    return {"chunks": chunks, "mixed_idx": mixed_idx, "nmix": len(order),
            "order": order}


def _prep_core_inputs(inputs, plan):
    """Per-core input dicts (8 cores: batch-major, then head-group)."""
    import ml_dtypes
    bf16 = ml_dtypes.bfloat16

    inputs_q = np.ascontiguousarray(inputs["inputs_q"], dtype=np.float32)
    inputs_kv = np.ascontiguousarray(inputs["inputs_kv"], dtype=np.float32)
    key_prefix = np.asarray(inputs["key_prefix"], dtype=np.float32)
    value_prefix = np.asarray(inputs["value_prefix"], dtype=np.float32)
    mask = np.asarray(inputs["mask"])
    Wq = np.asarray(inputs["Wq"], dtype=np.float32)
    Wk = np.asarray(inputs["Wk"], dtype=np.float32)
    Wv = np.asarray(inputs["Wv"], dtype=np.float32)
    Wo = np.asarray(inputs["Wo"], dtype=np.float32)

    def xprep(x):
        # [L, E] -> xT [E, L] -> [NG, 128, 8, 512] (partition-contiguous)
        return np.ascontiguousarray(
            x.T.reshape(8, 128, NG, 512).transpose(2, 1, 0, 3).astype(bf16))

    def wprep(w):
        # [E, M] -> [128, 8, M] (partition-contiguous)
        m = w.shape[1]
        return np.ascontiguousarray(
            w.reshape(8, 128, m).transpose(1, 0, 2).astype(bf16))

    xT = [xprep(inputs_q[b]) for b in range(B)]
    xkT = [xprep(inputs_kv[b]) for b in range(B)]

    maskblks = []
    for b in range(B):
        mb = np.zeros((max(plan["nmix"], 1), 128, 512), bf16)
        for i, (g, c) in enumerate(plan["order"]):
            jd = c - 1 - 4 * g
            if 1 <= jd <= 3:
                # only the in-block triangle (q rows 128*jd..128*jd+128)
                mb[i][:, 0:128] = mask[
                    b, 0, 512 * g + 128 * jd:512 * g + 128 * jd + 128,
                    128 * (c - 1):128 * c].T.astype(bf16)
            else:
                mb[i] = mask[b, 0, 512 * g:512 * g + 512,
                             128 * (c - 1):128 * c].T.astype(bf16)
        maskblks.append(mb)

    in_maps = []
    for core in range(NCORES):
        b, hg = core // HGROUPS, core % HGROUPS
        hs = slice(HPC * hg, HPC * (hg + 1))
        kpT = key_prefix[b, :, hs, :]                 # [P, 4, D]
        kpT = kpT.transpose(1, 2, 0).reshape(2, 128, P)  # [hc, (2 heads x D), P]
        kpT = np.concatenate(
            [kpT, np.zeros((2, 128, 128 - P), np.float32)], axis=2)
        kpT = np.ascontiguousarray(kpT.astype(bf16))
        im = {
            "xqT": xT[b],
            "xkvT": xkT[b],
            "wq": wprep((Wq[:, hs, :] / np.sqrt(D)).reshape(E, HPC * D)),
            "wk": wprep(Wk[:, hs, :].reshape(E, HPC * D)),
            "wv": wprep(Wv[:, hs, :].reshape(E, HPC * D)),
            "wo": np.ascontiguousarray(
                Wo[hs].reshape(2, 128, E).transpose(1, 0, 2).astype(bf16)),
            "kprefT": kpT,
            "vpref": np.ascontiguousarray(np.concatenate(
                [value_prefix[b, :, hs, :],
                 np.zeros((128 - P, HPC, D), np.float32)], axis=0).astype(bf16)),
        }
        if plan["nmix"]:
            im["maskblk"] = maskblks[b]
        in_maps.append(im)
    return in_maps


def kernel(**inputs) -> np.ndarray:
    from concourse import bass_utils

    plan = _make_plan(inputs["mask"])
    key = (tuple(tuple(c) for c in plan["chunks"]), tuple(plan["order"]))
    if key not in _CACHE:
        _CACHE[key] = _build_module(plan)
    nc = _CACHE[key]

    in_maps = _prep_core_inputs(inputs, plan)
    res = bass_utils.run_bass_kernel_spmd(nc, in_maps, core_ids=list(range(NCORES)))

    out = np.zeros((B, LQ, E), np.float32)
    for core in range(NCORES):
        b = core // HGROUPS
        out[b] += np.asarray(res.results[core]["outT"], dtype=np.float32).T
    return out


# revision 37
# speedup vs baseline: 1.0249x; 1.0249x over previous
"""Multi-head dot-product attention with prefix KV, on 8 trn2 NeuronCores.

Sharding: batch (2) x head-groups (4 groups of 4 heads) = 8 cores.
Each core computes q/k/v projections for its 4 heads, flash-style
attention (scores kept transposed: [kv, L] so no on-device transposes
are needed), and a partial out-projection [E, L]; the host sums the 4
head-group partials per batch and transposes back.

Device-side layout notes:
  - All matmul operands are bf16 (PSUM accumulation stays fp32): on HW,
    f32r lowers to fp32_mode=HIGH multi-pass matmuls (~2-4 cyc/row) and
    2-pass LDWEIGHTS; bf16 runs 1 cyc/row with half-cost weight loads.
  - Host pre-transposes inputs_q/inputs_kv to x^T [E, L] so both the
    qT/kT projections (lhsT=W) and the natural-layout v projection
    (lhsT=x^T tiles) need no on-device transposes.
  - kv axis is padded to 2176 = 17*128: chunk 0 = [prefix(64) | dead(64)],
    chunks 1..16 = kv positions.  Dead columns are killed with a
    per-partition -1e10 bias on the chunk-0 exp.
  - softmax runs without max subtraction (scores are O(1); masked
    entries underflow to exactly 0 like the reference's -1e10 bias).
  - denominator comes free as an extra ones-column in the v weights
    (M=65 ctx matmul); denominator rows of all 4 heads are gathered to
    partitions {0,32,64,96} of one tile, approx-reciprocal'd in one DVE
    op, and broadcast across partitions with K=1 outer-product matmuls.
  - q/k projections use 512-wide moving dims (1 PSUM bank) to amortize
    LDWEIGHTS; all projection / out-projection units are injected as PE
    filler BETWEEN the score matmuls and the ctx matmuls of every
    attention batch, covering the softmax (ACT) latency in the in-order
    PE queue for all four L-groups.
"""

import numpy as np

B, LQ, LKV, E, H, D, P = 2, 2048, 2048, 1024, 16, 64, 64
NCORES = 8
HGROUPS = 4          # head groups (cores per batch)
HPC = H // HGROUPS   # heads per core = 4
KVPAD = 128 + LKV    # 2176
NCH = KVPAD // 128   # 17 chunks
NG = LQ // 512       # 4 L-groups of 512
NEG = -1.0e10

_CACHE = {}


def _build_module(plan, debug_taps=False):
    """Build the single-core Bass module (same program for all 8 cores)."""
    import concourse.bass as bass
    import concourse.tile as tile
    import concourse.mybir as mybir
    from concourse import bacc
    from contextlib import ExitStack

    f32 = mybir.dt.float32
    bf16 = mybir.dt.bfloat16
    Exp = mybir.ActivationFunctionType.Exp

    chunks, mixed_idx, nmix = plan["chunks"], plan["mixed_idx"], plan["nmix"]

    nc = bacc.Bacc("TRN2", target_bir_lowering=False, debug=False,
                   enable_asserts=False, num_devices=NCORES)

    # x and weights are pre-arranged on the host so every big DMA is a
    # dense per-partition-contiguous 2D transfer (no strided gathers).
    xqT_d = nc.dram_tensor("xqT", [NG, 128, 8, 512], bf16, kind="ExternalInput").ap()
    xkvT_d = nc.dram_tensor("xkvT", [NG, 128, 8, 512], bf16, kind="ExternalInput").ap()
    wq_d = nc.dram_tensor("wq", [128, 8, 256], bf16, kind="ExternalInput").ap()
    wk_d = nc.dram_tensor("wk", [128, 8, 256], bf16, kind="ExternalInput").ap()
    wv_d = nc.dram_tensor("wv", [128, 8, 256], bf16, kind="ExternalInput").ap()
    wo_d = nc.dram_tensor("wo", [128, 2, 1024], bf16, kind="ExternalInput").ap()
    kprefT_d = nc.dram_tensor("kprefT", [2, 128, 128], bf16, kind="ExternalInput").ap()
    vpref_d = nc.dram_tensor("vpref", [128, HPC, D], bf16, kind="ExternalInput").ap()
    if nmix:
        maskblk_d = nc.dram_tensor("maskblk", [nmix, 128, 512], bf16,
                                   kind="ExternalInput").ap()
    # bf16 output partials: halves the store drain; the host sums the four
    # per-core partials in fp32 (adds ~1e-3 abs error, well inside budget).
    outT_d = nc.dram_tensor("outT", [E, LQ], bf16, kind="ExternalOutput").ap()

    with tile.TileContext(nc) as tc, ExitStack() as stk:
        pers = stk.enter_context(tc.tile_pool(name="pers", bufs=1))

        def ptile(shape, name, dt=None):
            return pers.tile(shape, dt or bf16, tag=name, name=name)

        wq_sb = ptile([128, 8, 256], "wq_sb")
        wk_sb = ptile([128, 8, 256], "wk_sb")
        wv_sb = ptile([128, 8, 256], "wv_sb")
        wo_sb = ptile([128, 2, 1024], "wo_sb")
        # per-slice tensors: QTS[hc][g] 512-wide; KTS[hc][s]: s=0 prefix
        # [128,128], s>=1 [128,256] (kv chunks 2s-1, 2s); VTS[c] per chunk.
        QTS = [[ptile([128, 512], f"QT{i}g{g}") for g in range(NG)] for i in range(2)]
        KTS = [[ptile([128, 128] if s == 0 else [128, 256], f"KT{i}s{s}")
                for s in range(9)] for i in range(2)]
        VTS = [ptile([128, HPC, 65], f"VT{c}") for c in range(NCH)]
        CTXT = [[ptile([128, 512], f"CTXT{i}g{g}") for g in range(NG)]
                for i in range(2)]
        cb0 = ptile([128, 1], "cb0", f32)
        ones_col = ptile([128, 64], "ones_col")

        def kslice(hc, c):
            if c == 0:
                return KTS[hc][0][:, 0:128]
            s, off = (c + 1) // 2, 128 * ((c - 1) % 2)
            return KTS[hc][s][:, off:off + 128]

        # wq goes on the scalar hardware DGE queue, in parallel with the
        # first x load on the sync queue; remaining weights follow after the
        # first x loads so the critical first projection isn't queued behind
        # 2MB of other transfers.
        nc.scalar.dma_start(out=wq_sb, in_=wq_d)

        def load_rest():
            nc.scalar.dma_start(out=wk_sb, in_=wk_d)
            nc.sync.dma_start(out=wv_sb, in_=wv_d)
            nc.scalar.dma_start(out=wo_sb, in_=wo_d)
            for hc in range(2):
                nc.sync.dma_start(out=KTS[hc][0], in_=kprefT_d[hc])
            nc.sync.dma_start(out=VTS[0][:, :, 0:D], in_=vpref_d)

        nc.vector.memset(cb0[0:64, :], 0.0)
        nc.vector.memset(cb0[64:128, :], NEG)
        nc.gpsimd.memset(ones_col, 1.0)
        for c in range(NCH):
            nc.gpsimd.memset(VTS[c][:, :, 64:65], 1.0)

        xio = stk.enter_context(tc.tile_pool(name="xio", bufs=2))
        attps = stk.enter_context(tc.tile_pool(name="att_ps", bufs=1, space="PSUM"))
        attsb = stk.enter_context(tc.tile_pool(name="att_sb", bufs=1))
        pp = stk.enter_context(tc.tile_pool(name="pp", bufs=1, space="PSUM"))

        def proj_load(g):
            xq_t = xio.tile([128, 8, 512], bf16, tag="xq", bufs=2, name="xq_t")
            xkv_t = xio.tile([128, 8, 512], bf16, tag="xkv", bufs=2, name="xkv_t")
            nc.sync.dma_start(out=xq_t, in_=xqT_d[g])
            nc.scalar.dma_start(out=xkv_t, in_=xkvT_d[g])
            return xq_t, xkv_t

        def q_unit(g, t, xq_t):
            """qT projection for 512-wide L-group g, 128-row chunk t."""
            ps_q = pp.tile([128, 512], f32, tag="pj", bufs=2, name="ps_q")
            for ec in range(8):
                nc.tensor.matmul(
                    ps_q, lhsT=wq_sb[:, ec, 128 * t:128 * t + 128],
                    rhs=xq_t[:, ec, :], start=(ec == 0), stop=(ec == 7))
            nc.vector.tensor_copy(out=QTS[t][g], in_=ps_q)

        def k_unit(g, t, xkv_t):
            ps_k = pp.tile([128, 512], f32, tag="pj", bufs=2, name="ps_k")
            for ec in range(8):
                nc.tensor.matmul(
                    ps_k, lhsT=wk_sb[:, ec, 128 * t:128 * t + 128],
                    rhs=xkv_t[:, ec, :], start=(ec == 0), stop=(ec == 7))
            nc.vector.tensor_copy(out=KTS[t][2 * g + 1], in_=ps_k[:, 0:256])
            nc.vector.tensor_copy(out=KTS[t][2 * g + 2], in_=ps_k[:, 256:512])

        def v_unit(g, sub, xkv_t):
            """v projection for kv chunk 4g+sub+1 (natural [kv, hd] layout)."""
            ps_v = pp.tile([128, 512], f32, tag="pj", bufs=2, name="ps_v")
            for ec in range(8):
                nc.tensor.matmul(
                    ps_v[:, 0:256], lhsT=xkv_t[:, ec, 128 * sub:128 * sub + 128],
                    rhs=wv_sb[:, ec, :], start=(ec == 0), stop=(ec == 7))
            nc.vector.tensor_copy(
                out=VTS[1 + 4 * g + sub][:, :, 0:D],
                in_=ps_v[:, 0:256].rearrange("p (h d) -> p h d", h=HPC))

        def proj_units(g, loaded, skip_v=False):
            """Generator of (pe_cost_us, closure) projection units for group g."""
            xq_t, xkv_t = loaded
            for t in range(2):
                yield (1.7, lambda t=t: q_unit(g, t, xq_t))
                yield (1.7, lambda t=t: k_unit(g, t, xkv_t))
            if not skip_v:
                for sub in range(4):
                    yield (0.85, lambda sub=sub: v_unit(g, sub, xkv_t))

        def outproj_unit(g, et):
            gl = 512 * g
            ops = pp.tile([128, 512], f32, tag="pj", bufs=2, name="ops")
            for hc in range(2):
                nc.tensor.matmul(
                    ops, lhsT=wo_sb[:, hc, 128 * et:128 * et + 128],
                    rhs=CTXT[hc][g], start=(hc == 0), stop=(hc == 1))
            ot = attsb.tile([128, 512], bf16, tag="ostage", bufs=3, name="ot")
            # during attention (g<3) ACT is busy with exp: keep copies on DVE
            # and stores on the sync queue.  In the tail (g==3) ACT is idle:
            # alternate both the copy engine and the DMA queue to halve the
            # final drain.
            if g < 3 or et % 2 == 0:
                nc.vector.tensor_copy(out=ot, in_=ops)
            else:
                nc.scalar.copy(ot, ops)
            eng = nc.sync if (g < 3 or et % 2 == 0) else nc.scalar
            eng.dma_start(
                out=outT_d[128 * et:128 * et + 128, gl:gl + 512], in_=ot)

        def outproj_units(g):
            for et in range(8):
                yield (0.5, lambda et=et: outproj_unit(g, et))

        def attn_group(g, mts, filler=None):
            def fill(target=1.0):
                if filler is None:
                    return
                acc = 0.0
                while acc < target:
                    try:
                        cost, fn = next(filler)
                    except StopIteration:
                        return
                    fn()
                    acc += cost

            cs = chunks[g]
            batches = [[cs[0]]] + [cs[1 + i:3 + i] for i in range(0, len(cs) - 1, 2)]
            nb = len(batches)
            ctxs = {}
            pend = []

            def qstart(c):
                """First valid q column (within the group's 512) for chunk c.
                Diagonal chunks with jd>=1 have no valid scores for q<128*jd:
                those columns are skipped in scores/exp/ctx instead of being
                masked (the jd==0 chunk keeps the full-width mask multiply
                for its in-block triangle)."""
                if (g, c) in mixed_idx:
                    jd = c - 1 - 4 * g
                    if 1 <= jd <= 3:
                        return 128 * jd
                return 0
            for hp in range(HPC // 2):
                heads = (2 * hp, 2 * hp + 1)
                ctx_ps = {}
                for h in heads:
                    ctx_ps[h] = attps.tile([65, 512], f32, tag="ctx", bufs=2,
                                           name=f"ctx{h}")

                def emit_scores(bi, batch):
                    """Scores + exp + mask for one batch; returns pr tiles."""
                    w = 512 * len(batch)
                    w0 = qstart(batch[0])
                    pr = {}
                    for h in heads:
                        sc = attps.tile([128, 1024], f32, tag="sc", bufs=2,
                                        name=f"sc{h}")
                        for j, c in enumerate(batch):
                            prow = 64 * (h % 2)
                            q0 = qstart(c)
                            nc.tensor.matmul(
                                sc[:, 512 * j + q0:512 * j + 512],
                                lhsT=kslice(hp, c)[prow:prow + 64, :],
                                rhs=QTS[hp][g][prow:prow + 64, q0:512],
                                start=True, stop=True)
                        pr[h] = attsb.tile([128, 1024], bf16, tag="probs",
                                           bufs=4, name=f"pr{h}")
                        if batch[0] == 0:
                            nc.scalar.activation(pr[h][:, 0:w], sc[:, 0:w],
                                                 Exp, bias=cb0[:, 0:1])
                        else:
                            nc.scalar.activation(pr[h][:, w0:w], sc[:, w0:w], Exp)
                        for j, c in enumerate(batch):
                            if (g, c) in mts:
                                q0 = qstart(c)
                                if q0 == 0:
                                    nc.vector.tensor_mul(
                                        pr[h][:, 512 * j:512 * j + 512],
                                        pr[h][:, 512 * j:512 * j + 512],
                                        mts[(g, c)])
                                else:
                                    # only the 128-wide in-block triangle
                                    # needs masking; columns < q0 are never
                                    # read and columns > q0+128 are all-valid
                                    o = 512 * j + q0
                                    nc.vector.tensor_mul(
                                        pr[h][:, o:o + 128],
                                        pr[h][:, o:o + 128], mts[(g, c)])
                    return pr

                def emit_ctx(bi, batch, pr):
                    for h in heads:
                        for j, c in enumerate(batch):
                            last = (bi == nb - 1 and j == len(batch) - 1)
                            q0 = qstart(c)
                            V = VTS[c][:, h % 2 + 2 * hp, :]
                            if q0 == 0:
                                nc.tensor.matmul(
                                    ctx_ps[h], lhsT=V,
                                    rhs=pr[h][:, 512 * j:512 * j + 512],
                                    start=(bi == 0 and j == 0), stop=last)
                            else:
                                o = 512 * j + q0
                                nc.tensor.matmul(
                                    ctx_ps[h][:, q0:q0 + 128], lhsT=V,
                                    rhs=pr[h][:, o:o + 128],
                                    start=False, stop=last and q0 == 384,
                                    skip_group_check=True)
                                if q0 + 128 < 512:
                                    nc.tensor.matmul(
                                        ctx_ps[h][:, q0 + 128:512], lhsT=V,
                                        rhs=pr[h][:, o + 128:512 * j + 512],
                                        start=False, stop=last,
                                        skip_group_check=True)

                # software-pipelined emission: scores of batch bi+1 sit in the
                # in-order PE queue BEFORE ctx of batch bi, so the PE streams
                # scores(bi+1) while exp(bi) runs on ACT — attention hides its
                # own softmax latency; filler is only topping up.
                prev = None
                for bi, batch in enumerate(batches):
                    pr = emit_scores(bi, batch)
                    fill(0.5)
                    if pend and bi == 1:
                        # inject the previous head-pair's broadcast+scale here:
                        # by now its DVE reciprocal chain has resolved, so the
                        # PE does not stall on it.
                        for fn in pend:
                            fn()
                        pend = []
                    if prev is not None:
                        emit_ctx(*prev)
                    prev = (bi, batch, pr)
                emit_ctx(*prev)
                # head-pair epilogue: drain ctx PSUM, then per-head in-place
                # approx-reciprocal of the denominator row at partition 64
                # (no cross-partition DMA needed); the PE-side broadcast+scale
                # is deferred (pend) into the next head-pair's batch stream.
                rcf = attsb.tile([65, 1024], f32, tag="rcf", bufs=2, name="rcf")
                rcb = attsb.tile([65, 1024], bf16, tag="rcb", bufs=2, name="rcb")
                for h in heads:
                    ctxs[h] = attsb.tile([65, 512], f32, tag="ctxs", bufs=4,
                                         name=f"ctxs{h}")
                    nc.vector.tensor_copy(out=ctxs[h], in_=ctx_ps[h])
                    hr = h % 2
                    # full-tile raf: custom-DVE uops misbehave on partition
                    # slices with non-zero base, and the cost is free-size
                    # based anyway; only row 64 (the denom) is meaningful.
                    nc.vector.reciprocal_approx_fast(
                        out=rcf[:, 512 * hr:512 * hr + 512], in_=ctxs[h])
                nc.vector.tensor_copy(out=rcb[64:65, :], in_=rcf[64:65, :])

                def bcast_scale(hp=hp, heads=heads, rcb=rcb):
                    for h in heads:
                        hr = h % 2
                        bc_ps = pp.tile([64, 512], f32, tag="pj", bufs=2,
                                        name="bc_ps")
                        nc.tensor.matmul(bc_ps,
                                         lhsT=ones_col[64:65, :],
                                         rhs=rcb[64:65, 512 * hr:512 * hr + 512],
                                         start=True, stop=True,
                                         tile_position=(64, 0))
                        if hr == 0:
                            nc.vector.tensor_mul(CTXT[hp][g][0:64, :],
                                                 ctxs[h][0:64, :], bc_ps)
                        else:
                            st = attsb.tile([64, 512], bf16, tag="stage", bufs=2,
                                            name="st")
                            nc.vector.tensor_mul(st, ctxs[h][0:64, :], bc_ps)
                            # scalar queue: this partition move gates the
                            # group's out-projection, so don't queue it
                            # behind pending output stores on sync
                            nc.scalar.dma_start(out=CTXT[hp][g][64:128, :], in_=st)
                pend.append(bcast_scale)
            # last head-pair's chain: give the PE filler work while the DVE
            # reciprocal resolves, then emit the broadcast+scale.
            fill(3.0)
            for fn in pend:
                fn()

        # mask tiles loaded lazily per group so their DMA dispatches don't
        # delay the first x loads on the in-order sync engine.
        mts = {}

        def load_masks(g):
            for c in chunks[g]:
                if (g, c) in mixed_idx:
                    jd = c - 1 - 4 * g
                    if 1 <= jd <= 3:
                        # only the 128-wide in-block triangle is needed
                        mt = attsb.tile([128, 128], bf16, tag="mask",
                                        bufs=max(nmix, 1), name=f"mt{g}_{c}")
                        nc.sync.dma_start(
                            out=mt, in_=maskblk_d[mixed_idx[(g, c)]][:, 0:128])
                    else:
                        mt = attsb.tile([128, 512], bf16, tag="mask",
                                        bufs=max(nmix, 1), name=f"mt{g}_{c}")
                        nc.sync.dma_start(out=mt, in_=maskblk_d[mixed_idx[(g, c)]])
                    mts[(g, c)] = mt

        # interleaved schedule: group-g attention is fed PE-filler units
        # (projections of group g+1, then out-projections of finished
        # groups) between its scores and ctx matmuls.  Fill demand per
        # group is 2*len(batches)+2 pulls = 8/12/16/20; supply is matched
        # where possible.
        def chain(*gens):
            for gn in gens:
                yield from gn

        ld0 = proj_load(0)
        load_rest()
        load_masks(0)
        for _, fn in proj_units(0, ld0):
            fn()
        ld1 = proj_load(1)
        load_masks(1)
        f1 = proj_units(1, ld1)
        attn_group(0, mts, filler=f1)
        for _, fn in f1:
            fn()
        ld2 = proj_load(2)
        load_masks(2)
        op0 = list(outproj_units(0))
        op1 = list(outproj_units(1))
        f2 = chain(proj_units(2, ld2), iter(op0[:4]))
        attn_group(1, mts, filler=f2)
        for _, fn in f2:
            fn()
        ld3 = proj_load(3)
        load_masks(3)
        f3 = chain(iter(op0[4:]), proj_units(3, ld3, skip_v=True), iter(op1[:4]))
        attn_group(2, mts, filler=f3)
        for _, fn in f3:
            fn()
        xkv3 = ld3[1]
        f4 = chain(((0.85, lambda sub=sub: v_unit(3, sub, xkv3))
                    for sub in range(4)),
                   iter(op1[4:]), outproj_units(2))
        attn_group(3, mts, filler=f4)
        for _, fn in f4:
            fn()
        for _, fn in outproj_units(3):
            fn()

    nc.compile()
    return nc


def _make_plan(mask):
    """Block plan from the actual mask (union over batches -> one SPMD plan)."""
    m = np.asarray(mask[:, 0])                       # [B, LQ, LKV] bool
    blk = m.reshape(B, NG, 512, LKV // 128, 128)
    any_b = blk.any(axis=(2, 4)).any(axis=0)         # [NG, 16]
    all_b = blk.all(axis=(2, 4)).all(axis=0)         # [NG, 16]
    chunks, mixed_idx = [], {}
    order = []
    for g in range(NG):
        cl = [0]
        for c in range(1, NCH):
            if any_b[g, c - 1]:
                cl.append(c)
                if not all_b[g, c - 1]:
                    mixed_idx[(g, c)] = len(order)
                    order.append((g, c))
        chunks.append(cl)
    return {"chunks": chunks, "mixed_idx": mixed_idx, "nmix": len(order),
            "order": order}


def _prep_core_inputs(inputs, plan):
    """Per-core input dicts (8 cores: batch-major, then head-group)."""
    import ml_dtypes
    bf16 = ml_dtypes.bfloat16

    inputs_q = np.ascontiguousarray(inputs["inputs_q"], dtype=np.float32)
    inputs_kv = np.ascontiguousarray(inputs["inputs_kv"], dtype=np.float32)
    key_prefix = np.asarray(inputs["key_prefix"], dtype=np.float32)
    value_prefix = np.asarray(inputs["value_prefix"], dtype=np.float32)
    mask = np.asarray(inputs["mask"])
    Wq = np.asarray(inputs["Wq"], dtype=np.float32)
    Wk = np.asarray(inputs["Wk"], dtype=np.float32)
    Wv = np.asarray(inputs["Wv"], dtype=np.float32)
    Wo = np.asarray(inputs["Wo"], dtype=np.float32)

    def xprep(x):
        # [L, E] -> xT [E, L] -> [NG, 128, 8, 512] (partition-contiguous)
        return np.ascontiguousarray(
            x.T.reshape(8, 128, NG, 512).transpose(2, 1, 0, 3).astype(bf16))

    def wprep(w):
        # [E, M] -> [128, 8, M] (partition-contiguous)
        m = w.shape[1]
        return np.ascontiguousarray(
            w.reshape(8, 128, m).transpose(1, 0, 2).astype(bf16))

    xT = [xprep(inputs_q[b]) for b in range(B)]
    xkT = [xprep(inputs_kv[b]) for b in range(B)]

    maskblks = []
    for b in range(B):
        mb = np.zeros((max(plan["nmix"], 1), 128, 512), bf16)
        for i, (g, c) in enumerate(plan["order"]):
            jd = c - 1 - 4 * g
            if 1 <= jd <= 3:
                # only the in-block triangle (q rows 128*jd..128*jd+128)
                mb[i][:, 0:128] = mask[
                    b, 0, 512 * g + 128 * jd:512 * g + 128 * jd + 128,
                    128 * (c - 1):128 * c].T.astype(bf16)
            else:
                mb[i] = mask[b, 0, 512 * g:512 * g + 512,
                             128 * (c - 1):128 * c].T.astype(bf16)
        maskblks.append(mb)

    in_maps = []
    for core in range(NCORES):
        b, hg = core // HGROUPS, core % HGROUPS
        hs = slice(HPC * hg, HPC * (hg + 1))
        kpT = key_prefix[b, :, hs, :]                 # [P, 4, D]
        kpT = kpT.transpose(1, 2, 0).reshape(2, 128, P)  # [hc, (2 heads x D), P]
        kpT = np.concatenate(
            [kpT, np.zeros((2, 128, 128 - P), np.float32)], axis=2)
        kpT = np.ascontiguousarray(kpT.astype(bf16))
        im = {
            "xqT": xT[b],
            "xkvT": xkT[b],
            "wq": wprep((Wq[:, hs, :] / np.sqrt(D)).reshape(E, HPC * D)),
            "wk": wprep(Wk[:, hs, :].reshape(E, HPC * D)),
            "wv": wprep(Wv[:, hs, :].reshape(E, HPC * D)),
            "wo": np.ascontiguousarray(
                Wo[hs].reshape(2, 128, E).transpose(1, 0, 2).astype(bf16)),
            "kprefT": kpT,
            "vpref": np.ascontiguousarray(np.concatenate(
                [value_prefix[b, :, hs, :],
                 np.zeros((128 - P, HPC, D), np.float32)], axis=0).astype(bf16)),
        }
        if plan["nmix"]:
            im["maskblk"] = maskblks[b]
        in_maps.append(im)
    return in_maps


def kernel(**inputs) -> np.ndarray:
    from concourse import bass_utils

    plan = _make_plan(inputs["mask"])
    key = (tuple(tuple(c) for c in plan["chunks"]), tuple(plan["order"]))
    if key not in _CACHE:
        _CACHE[key] = _build_module(plan)
    nc = _CACHE[key]

    in_maps = _prep_core_inputs(inputs, plan)
    res = bass_utils.run_bass_kernel_spmd(nc, in_maps, core_ids=list(range(NCORES)))

    out = np.zeros((B, LQ, E), np.float32)
    for core in range(NCORES):
        b = core // HGROUPS
        out[b] += np.asarray(res.results[core]["outT"], dtype=np.float32).T
    return out
